# revision 1
# baseline (speedup 1.0000x reference)
"""Causal GQA self-attention (B=2, S=2048, D=2048, 16 heads / 4 KV heads) on 8
Trainium2 NeuronCores.

Sharding: tensor-parallel over heads. Core c owns Q heads (2c, 2c+1) and KV
head c//2. Each core computes its heads' attention output and a partial
output projection (columns of Wp.T owned by its heads); the host sums the 8
partial outputs.

Device-side layout notes:
  - x is pre-transposed on host to xT [D, B*S] so every matmul contracts on
    the SBUF partition dim.
  - q/k are projected per 128-token tile into [token, feat] layout (natural),
    RMS-normed + RoPE'd along the free dim, then PE-transposed into
    qT/kT [head_dim, token] for the attention matmuls. v stays natural.
  - Causal attention is blocked (128 q x 512 k chunks) with the staircase
    padded to 512-column chunks; padding is masked to -1e30 pre-softmax.
  - attn tiles are PE-transposed (with identity) to [key, query] so the
    AV matmul contracts keys on partitions; 1/l is applied to attn rows
    before the transpose.
  - Matmuls run in float32r (full PE rate at moving-dim >= 256). float32r
    operands come either straight from DMA (f32r-declared DRAM tensors) or
    from ACT/DVE rounding copies.
"""

import math

import numpy as np

B = 2
S = 2048
D = 2048
T = B * S
NH = 16
NKV = 4
HD = 128
P = 128
ROPE_BASE = 10000.0
EPS = float(np.finfo(np.float32).eps)
NEG = -1.0e30

N_CORES = 8
TT_B = S // P          # 16 token tiles per batch
GROUPS = 4             # groups of 4 q-tiles (512 queries)
QKV = 512              # per-core fused projection width: 2*q + k + v heads

_PROG = {}


def _build_program(loop_n=0, phases=("qkv", "attn", "proj")):
    import concourse.mybir as mybir
    import concourse.tile as tile
    from concourse import bacc
    from concourse.masks import make_identity

    f32 = mybir.dt.float32
    f32r = mybir.dt.float32r
    AL = mybir.AluOpType
    AF = mybir.ActivationFunctionType
    AX = mybir.AxisListType

    nc = bacc.Bacc("TRN2", target_bir_lowering=False, debug=False,
                   enable_asserts=True, num_devices=N_CORES)

    xT = nc.dram_tensor("xT", [D, T], f32r, kind="ExternalInput").ap()
    wcat = nc.dram_tensor("wcat", [D, QKV], f32r, kind="ExternalInput").ap()
    wp = nc.dram_tensor("wp", [2 * HD, D], f32r, kind="ExternalInput").ap()
    cosd = nc.dram_tensor("cosd", [T, HD // 2], f32, kind="ExternalInput").ap()
    sind = nc.dram_tensor("sind", [T, HD // 2], f32, kind="ExternalInput").ap()
    maskd = nc.dram_tensor("maskd", [P, 896], f32, kind="ExternalInput").ap()
    gaind = nc.dram_tensor("gaind", [P, 4], f32, kind="ExternalInput").ap()
    outd = nc.dram_tensor("out", [T, D], f32, kind="ExternalOutput").ap()

    xT_r = xT.rearrange("(kt p) t -> p kt t", p=P)        # [128, 16, T]
    wcat_r = wcat.rearrange("(kt p) n -> p kt n", p=P)    # [128, 16, 512]
    wp_r = wp.rearrange("(ct p) o -> p ct o", p=P)        # [128, 2, D]


    import contextlib as _ctxlib
    with tile.TileContext(nc) as tc, _ctxlib.ExitStack() as _es:
        pc = _es.enter_context(tc.tile_pool(name="const", bufs=1))
        pb = _es.enter_context(tc.tile_pool(name="batch", bufs=1))
        px = _es.enter_context(tc.tile_pool(name="xs", bufs=2))
        pw = _es.enter_context(tc.tile_pool(name="work", bufs=2))
        pat = _es.enter_context(tc.tile_pool(name="attn", bufs=1))
        psm = _es.enter_context(tc.tile_pool(name="small", bufs=4))
        po = _es.enter_context(tc.tile_pool(name="outp", bufs=3))
        prl = _es.enter_context(tc.tile_pool(name="rlp", bufs=2))
        prq = _es.enter_context(tc.tile_pool(name="rlq", bufs=1))
        pcs = _es.enter_context(tc.tile_pool(name="cs2", bufs=2))
        pat2 = _es.enter_context(tc.tile_pool(name="at3", bufs=3))
        ppA = _es.enter_context(tc.tile_pool(name="psA", bufs=2, space="PSUM"))
        ppB = _es.enter_context(tc.tile_pool(name="psB", bufs=3, space="PSUM"))
        ppC = _es.enter_context(tc.tile_pool(name="psC", bufs=1, space="PSUM"))
        # ---- constants resident in SBUF
        wcat_sb = pc.tile([P, TT_B, QKV], f32r, tag="wcat")
        for kt in range(TT_B):
            nc.sync.dma_start(wcat_sb[:, kt, :], wcat_r[:, kt, :])
        wp_sb = pc.tile([P, 2, D], f32r, tag="wp")
        nc.sync.dma_start(wp_sb[:], wp_r[:])
        mask_sb = pc.tile([P, 896], f32, tag="mask")
        nc.sync.dma_start(mask_sb[:], maskd[:])
        gain_sb = pc.tile([P, 4], f32, tag="gain")
        nc.sync.dma_start(gain_sb[:], gaind[:])
        idf = pc.tile([P, P], f32, tag="idf")
        make_identity(nc, idf[:])
        idr = pc.tile([P, P], f32r, tag="idr")
        nc.vector.tensor_copy(idr[:], idf[:])

        def psum_tile(shape, tag, dt_=f32):
            pool = {"sc": ppB, "av": ppC}.get(tag, ppA)
            return pool.tile(shape, dt_, tag=tag, name=tag)

        import contextlib
        loop_cm = contextlib.nullcontext()
        with loop_cm:
          for b in [bb % B for bb in range(B * max(1, loop_n))]:
              qT_h = [pb.tile([P, 2, S // 2], f32r, tag="qTlo", name="qTlo"),
                      pb.tile([P, 2, S // 2], f32r, tag="qThi", name="qThi")]
              kT_h = [pb.tile([P, S // 2], f32r, tag="kTlo", name="kTlo"),
                      pb.tile([P, S // 2], f32r, tag="kThi", name="kThi")]
              vN_h = [pb.tile([P, TT_B // 2, HD], f32r, tag="vNlo", name="vNlo"),
                      pb.tile([P, TT_B // 2, HD], f32r, tag="vNhi", name="vNhi")]

              def qT_at(hh, it):
                  half_i, loc = divmod(it, TT_B // 2)
                  return qT_h[half_i][:, hh, loc * P:(loc + 1) * P]

              def kT_rng(c0, w):
                  half_i, loc = divmod(c0, S // 2)
                  return kT_h[half_i][:, loc:loc + w]

              def vN_at(jt):
                  half_i, loc = divmod(jt, TT_B // 2)
                  return vN_h[half_i][:, loc, :]
              yT = pb.tile([P, 2, S], f32r, tag="yT")

              # ================= QKV projection + RMS + RoPE =================
              if "qkv" not in phases:
                  for tl in qT_h + kT_h + vN_h:
                      nc.vector.memset(tl[:], 0.0)
              nt_qkv = TT_B if "qkv" in phases else 0
              HB = 8
              for half in range(0, nt_qkv, HB):
                stgs = {}
                ssq_all = pb.tile([P, HB, 3], f32, tag="ssq_all")
                for tt in range(half, half + HB):
                  t0 = b * S + tt * P
                  xt = px.tile([P, TT_B, P], f32r, tag="xt")
                  nc.sync.dma_start(xt[:], xT_r[:, :, t0:t0 + P])

                  pp = psum_tile([P, QKV], "pmm")
                  for kt in range(TT_B):
                      nc.tensor.matmul(pp[:], xt[:, kt, :], wcat_sb[:, kt, :],
                                       start=(kt == 0), stop=(kt == TT_B - 1))

                  # v: rounding copy straight out of PSUM
                  nc.scalar.copy(vN_at(tt), pp[:, 3 * HD:4 * HD])
                  # stage q0,q1,k in SBUF; sum-of-squares per segment
                  stg = pb.tile([P, 3 * HD], f32, tag=f"stg{tt % HB}")
                  stgs[tt] = stg
                  nc.scalar.copy(stg[:], pp[:, :3 * HD])
                  scr = prq.tile([P, 3 * HD], f32, tag="scr")
                  nc.vector.tensor_tensor(scr[:], stg[:], stg[:], AL.mult)
                  nc.vector.tensor_reduce(
                      ssq_all[:, tt % HB, :], scr[:].rearrange("p (s x) -> p s x", s=3),
                      axis=AX.X, op=AL.add)

                # batched rsqrt: rs = exp(-0.5*ln(ssq/HD+eps)) * gain
                lnb = pb.tile([P, HB, 3], f32, tag="lnb")
                nc.scalar.activation(lnb[:], ssq_all[:], AF.Ln,
                                     scale=1.0 / HD, bias=gain_sb[:, 3:4])
                rsb = pb.tile([P, HB, 3], f32, tag="rsb")
                nc.scalar.activation(rsb[:], lnb[:], AF.Exp, scale=-0.5)
                rsg = pb.tile([P, HB, 3], f32, tag="rsg")
                nc.vector.tensor_tensor(
                    rsg[:], rsb[:],
                    gain_sb[:, None, :3].to_broadcast([P, HB, 3]), AL.mult)

                for tt in range(half, half + HB):
                  t0 = b * S + tt * P
                  cosb = pcs.tile([P, HD // 2], f32, tag="cosb")
                  nc.sync.dma_start(cosb[:], cosd[t0:t0 + P, :])
                  sinb = pcs.tile([P, HD // 2], f32, tag="sinb")
                  nc.sync.dma_start(sinb[:], sind[t0:t0 + P, :])
                  ppv = stgs[tt][:].rearrange("p (s x) -> p s x", s=3)

                  qn = pw.tile([P, 3, HD], f32, tag="qn")
                  nc.vector.tensor_tensor(
                      qn[:], ppv,
                      rsg[:, tt % HB, :, None].to_broadcast([P, 3, HD]), AL.mult)

                  # rope: out1 = a*cos + b2*sin ; out2 = b2*cos - a*sin
                  a = qn[:, :, :HD // 2]
                  b2 = qn[:, :, HD // 2:]
                  rp = pw.tile([P, 3, HD], f32, tag="rp")
                  o1 = rp[:, :, :HD // 2]
                  o2 = rp[:, :, HD // 2:]
                  tmp = pw.tile([P, 3, HD // 2], f32, tag="ropetmp")
                  cb = cosb[:, None, :].to_broadcast([P, 3, HD // 2])
                  sb_ = sinb[:, None, :].to_broadcast([P, 3, HD // 2])
                  nc.gpsimd.tensor_tensor(o1, a, cb, AL.mult)
                  nc.vector.tensor_tensor(tmp[:], b2, sb_, AL.mult)
                  nc.vector.tensor_tensor(o1, o1, tmp[:], AL.add)
                  nc.gpsimd.tensor_tensor(o2, b2, cb, AL.mult)
                  nc.gpsimd.tensor_tensor(tmp[:], a, sb_, AL.mult)
                  nc.vector.tensor_tensor(o2, o2, tmp[:], AL.subtract)

                  # transpose q0,q1,k into [head_dim, token] and round to f32r
                  rpf = rp[:].rearrange("p s x -> p (s x)")
                  ptq = psum_tile([P, 512], "ptt")
                  for sseg in range(3):
                      nc.tensor.transpose(ptq[:, sseg * P:(sseg + 1) * P],
                                          rpf[:, sseg * P:(sseg + 1) * P], idf[:])
                  half_i, loc = divmod(tt, TT_B // 2)
                  nc.scalar.copy(
                      qT_h[half_i][:, :, loc * P:(loc + 1) * P],
                      ptq[:, :2 * P].rearrange("p (h x) -> p h x", h=2))
                  nc.vector.tensor_copy(
                      kT_h[half_i][:, loc * P:(loc + 1) * P], ptq[:, 2 * P:3 * P])

              # ================= attention (+ interleaved out-proj) =========
              if "attn" not in phases:
                  nc.vector.memset(yT[:], 0.0)
              zb = None
              if "proj" not in phases:
                  zb = po.tile([P, 512], f32, tag="zb")
                  nc.vector.memset(zb[:], 0.0)

              def proj_block(tt_list):
                  for tt in tt_list:
                      for oc in range(4):
                          if "proj" not in phases:
                              nc.sync.dma_start(
                                  outd[b * S + tt * P: b * S + (tt + 1) * P,
                                       oc * 512:(oc + 1) * 512], zb[:])
                              continue
                          pout = psum_tile([P, 512], "pmm")
                          for ct in range(2):
                              nc.tensor.matmul(
                                  pout[:], yT[:, ct, tt * P:(tt + 1) * P],
                                  wp_sb[:, ct, oc * 512:(oc + 1) * 512],
                                  start=(ct == 0), stop=(ct == 1))
                          ob = po.tile([P, 512], f32, tag="ob")
                          if (tt + oc) % 2 == 0:
                              nc.vector.tensor_copy(ob[:], pout[:])
                          else:
                              nc.scalar.copy(ob[:], pout[:])
                          nc.sync.dma_start(
                              outd[b * S + tt * P: b * S + (tt + 1) * P,
                                   oc * 512:(oc + 1) * 512], ob[:])

              for g in range(GROUPS if "attn" in phases else 0):
                  nj = 4 * (g + 1)       # key tiles in this group's staircase
                  for h in range(2):
                      attnT = pat.tile([P, TT_B, 512], f32r, tag="attnT")
                      rl4 = psm.tile([P, 4], f32, tag="rl4")
                      for il in range(4):
                          it = 4 * g + il
                          q_lhs = qT_at(h, it)
                          at = pat2.tile([P, S], f32r, tag="at")
                          lsums = psm.tile([P, GROUPS], f32, tag="lsums")
                          for cc in range(g + 1):
                              wcc = 512 if cc < g else (il + 1) * P
                              sc = psum_tile([P, 512], "sc")
                              nc.tensor.matmul(sc[:, :wcc], q_lhs,
                                               kT_rng(cc * 512, wcc),
                                               start=True, stop=True)
                              if cc == g:
                                  nc.vector.tensor_tensor(
                                      sc[:, il * P:(il + 1) * P],
                                      sc[:, il * P:(il + 1) * P],
                                      mask_sb[:, 384:512], AL.add)
                              nc.scalar.activation(
                                  at[:, cc * 512:cc * 512 + wcc], sc[:, :wcc], AF.Exp,
                                  accum_out=lsums[:, cc:cc + 1])
                          ltot = psm.tile([P, 1], f32, tag="ltot")
                          nc.vector.tensor_reduce(
                              ltot[:], lsums[:, :g + 1], axis=AX.X, op=AL.add)
                          nc.vector.reciprocal(rl4[:, il:il + 1], ltot[:])

                          # transpose valid attn tiles -> attnT[key, query]
                          for j0 in range(0, it + 1, 4):
                              jn = min(4, it + 1 - j0)
                              ptt = psum_tile([P, 512], "ptt", f32r)
                              for jj in range(jn):
                                  jt = j0 + jj
                                  nc.tensor.transpose(
                                      ptt[:, jj * P:(jj + 1) * P],
                                      at[:, jt * P:(jt + 1) * P], idr[:])
                              dst = attnT[:, j0:j0 + jn, il * P:(il + 1) * P]
                              src = ptt[:, :jn * P].rearrange("p (j x) -> p j x", j=jn)
                              nc.vector.tensor_copy(dst, src)
                          # zero the not-computed (above-diagonal) attnT region
                          if it + 1 < nj:
                              nzj = nj - (it + 1)
                              nc.vector.tensor_copy(
                                  attnT[:, it + 1:nj, il * P:(il + 1) * P],
                                  mask_sb[:, :nzj * P].rearrange("p (j x) -> p j x", j=nzj))

                      # 1/l broadcast to [128, 512]: transpose rl4 then fan out
                      ptl = psum_tile([P, 512], "ptt")
                      nc.tensor.transpose(ptl[:4, :P], rl4[:], idf[:])
                      rl4T = prq.tile([4, P], f32, tag="rl4T")
                      nc.vector.tensor_copy(rl4T[:], ptl[:4, :P])
                      rlv = prq.tile([1, 512], f32, tag="rlv")
                      for il in range(4):
                          nc.sync.dma_start(rlv[0:1, il * P:(il + 1) * P],
                                            rl4T[il:il + 1, :])
                      rlb = prl.tile([P, 512], f32, tag="rlb")
                      nc.gpsimd.partition_broadcast(rlb[:], rlv[0:1, :])

                      # AV: yT[e, 512 queries], normalized by 1/l on evac
                      ya = psum_tile([P, 512], "av")
                      for jt in range(nj):
                          nc.tensor.matmul(ya[:], vN_at(jt), attnT[:, jt, :],
                                           start=(jt == 0), stop=(jt == nj - 1))
                      nc.vector.tensor_tensor(
                          yT[:, h, g * 512:(g + 1) * 512], ya[:], rlb[:], AL.mult)

                  # out-proj for this group's tokens, overlapped with next group
                  proj_block(range(4 * g, 4 * g + 4))

              if "attn" not in phases:
                  proj_block(range(TT_B))

    nc.compile()
    return nc


def _get_program(loop_n=0, phases=("qkv", "attn", "proj")):
    key = (loop_n, tuple(phases))
    if key not in _PROG:
        _PROG[key] = _build_program(loop_n, phases)
    return _PROG[key]


def _host_prep(x, Wq, Wk, Wv, Wp, q_gain):
    """Build the 8 per-core input maps."""
    x = np.ascontiguousarray(x.reshape(T, D), dtype=np.float32)
    xT = np.ascontiguousarray(x.T)                       # [D, T]

    inv_freq = 1.0 / (ROPE_BASE ** (np.arange(0, HD, 2, dtype=np.float32) / HD))
    freqs = np.arange(S, dtype=np.float32)[:, None] * inv_freq[None, :]
    cos = np.cos(freqs).astype(np.float32)
    sin = np.sin(freqs).astype(np.float32)
    cosT = np.ascontiguousarray(np.tile(cos, (B, 1)))    # [T, 64]
    sinT = np.ascontiguousarray(np.tile(sin, (B, 1)))

    r = np.arange(P)[:, None]
    k = np.arange(896)[None, :]
    masks = np.where(k <= 384 + r, 0.0, NEG).astype(np.float32)   # [128, 896]

    in_maps = []
    for core in range(N_CORES):
        h0 = 2 * core
        kv = core // 2
        WqT = Wq[h0 * HD:(h0 + 2) * HD, :].T             # [D, 256]
        WkT = Wk[kv * HD:(kv + 1) * HD, :].T             # [D, 128]
        WvT = Wv[kv * HD:(kv + 1) * HD, :].T             # [D, 128]
        wcat = np.ascontiguousarray(
            np.concatenate([WqT, WkT, WvT], axis=1), dtype=np.float32)
        wpT = np.ascontiguousarray(
            Wp[:, h0 * HD:(h0 + 2) * HD].T, dtype=np.float32)   # [256, D]
        scale = 1.0 / math.sqrt(HD)
        gain = np.tile(np.array(
            [[q_gain[h0] * scale, q_gain[h0 + 1] * scale, 1.0, EPS]],
            dtype=np.float32), (P, 1))
        in_maps.append({
            "xT": xT,
            "wcat": wcat,
            "wp": wpT,
            "cosd": cosT,
            "sind": sinT,
            "maskd": masks,
            "gaind": np.ascontiguousarray(gain),
        })
    return in_maps


def kernel(x, Wq, Wk, Wv, Wp, q_gain):
    from concourse.bass_utils import run_bass_kernel_spmd

    nc = _get_program()
    in_maps = _host_prep(x, Wq, Wk, Wv, Wp, q_gain)
    try:
        res = run_bass_kernel_spmd(nc, in_maps, core_ids=list(range(N_CORES)))
    except Exception:
        # one retry: a previous crashed run can leave the exec unit wedged
        res = run_bass_kernel_spmd(nc, in_maps, core_ids=list(range(N_CORES)))
    total = np.zeros((T, D), dtype=np.float32)
    for r in res.results:
        total += r["out"]
    return total.reshape(B, S, D)



# revision 5
# speedup vs baseline: 1.0101x; 1.0101x over previous
"""Causal GQA self-attention (B=2, S=2048, D=2048, 16 heads / 4 KV heads) on 8
Trainium2 NeuronCores.

Sharding: (batch, kv-head). Core c owns batch c//4 and KV head c%4, plus that
KV head's 4 query heads. Each core computes the full attention for its
(batch, kv-group) and a partial output projection over its heads' 512
y-dims; the host sums the 4 partial outputs per batch.

Device-side layout:
  - x is pre-transposed on host to xT [D, S_local]; the fused QKV projection
    runs in token-natural layout ([tok, 512 q | 128 k | 128 v]) contracting
    over D on partitions.
  - q/k are RMS-normed + RoPE'd in natural layout (free-dim math), rounded
    to bf16, then PE-transposed (bf16 identity, 1.0 cyc/row) into
    qT [hd, tok] / kT [hd, tok]. v stays natural in bf16.
  - Scores are computed TRANSPOSED: scT[k, q] = kT_tile^T @ qT (contraction
    over hd on partitions), so exp tiles feed the AV matmul directly with
    no per-tile PE transposes of the attention matrix.
  - Flash-style streaming: per (head, q-group of 512), k-tiles of 128 are
    processed with a 3-tile software pipeline: sc matmul -> ACT exp (bf16)
    [-> DVE causal mask mult on diagonal tiles] -> {ones-matmul accumulating
    the softmax denominator in PSUM f32, AV matmul accumulating y}.
  - 1/l is broadcast over partitions (Pool) and applied on AV evacuation.
  - Out-projection (bf16) is interleaved into the PE stream two k-tiles
    after each q-group completes, keeping the tensor engine dense so it
    holds the 2.4 GHz p-state.
"""

import math

import numpy as np

B = 2
S = 2048
D = 2048
T = B * S
NH = 16
NKV = 4
HD = 128
P = 128
ROPE_BASE = 10000.0
EPS = float(np.finfo(np.float32).eps)

N_CORES = 8
TT = S // P            # 16 token tiles per core (one batch)
GROUPS = 4             # q-groups of 512 queries
QKV = 768              # fused projection width: 4*q + k + v
SCALE = 1.0 / math.sqrt(HD)

_PROG = {}


def _build_program(loop_n=0):
    import concourse.mybir as mybir
    import concourse.tile as tile
    from concourse import bacc
    from concourse.masks import make_identity

    f32 = mybir.dt.float32
    f32r = mybir.dt.float32r
    bf16 = mybir.dt.bfloat16
    AL = mybir.AluOpType
    AF = mybir.ActivationFunctionType
    AX = mybir.AxisListType

    nc = bacc.Bacc("TRN2", target_bir_lowering=False, debug=False,
                   enable_asserts=True, num_devices=N_CORES)

    xT = nc.dram_tensor("xT", [D, S], f32r, kind="ExternalInput").ap()
    wcat = nc.dram_tensor("wcat", [D, QKV], f32r, kind="ExternalInput").ap()
    wpd = nc.dram_tensor("wpd", [4 * HD, D], bf16, kind="ExternalInput").ap()
    cosd = nc.dram_tensor("cosd", [S, HD // 2], f32, kind="ExternalInput").ap()
    sind = nc.dram_tensor("sind", [S, HD // 2], f32, kind="ExternalInput").ap()
    maskd = nc.dram_tensor("maskd", [P, 4, 512], bf16, kind="ExternalInput").ap()
    gaind = nc.dram_tensor("gaind", [P, 6], f32, kind="ExternalInput").ap()
    outd = nc.dram_tensor("out", [S, D], f32, kind="ExternalOutput").ap()

    xT_r = xT.rearrange("(kt p) t -> p kt t", p=P)        # [128, 16, S]
    wcat_r = wcat.rearrange("(kt p) n -> p kt n", p=P)    # [128, 16, 768]
    wp_r = wpd.rearrange("(ct p) o -> p ct o", p=P)       # [128, 4, D]
    cos_r = cosd.rearrange("(tt p) f -> p tt f", p=P)     # [128, 16, 64]
    sin_r = sind.rearrange("(tt p) f -> p tt f", p=P)

    import contextlib as _ctxlib
    with tile.TileContext(nc) as tc, _ctxlib.ExitStack() as _es:
        pc = _es.enter_context(tc.tile_pool(name="const", bufs=1))
        pb = _es.enter_context(tc.tile_pool(name="batch", bufs=1))
        px = _es.enter_context(tc.tile_pool(name="xs", bufs=3))
        pstg = _es.enter_context(tc.tile_pool(name="stg", bufs=3))
        pscr = _es.enter_context(tc.tile_pool(name="scr", bufs=2))
        psml = _es.enter_context(tc.tile_pool(name="small", bufs=3))
        pqn = _es.enter_context(tc.tile_pool(name="qn", bufs=2))
        ptm = _es.enter_context(tc.tile_pool(name="ropetmp", bufs=2))
        prp = _es.enter_context(tc.tile_pool(name="rp", bufs=4))
        pep = _es.enter_context(tc.tile_pool(name="ep", bufs=6))
        pyT = _es.enter_context(tc.tile_pool(name="yT", bufs=2))
        prl = _es.enter_context(tc.tile_pool(name="rl", bufs=2))
        pob = _es.enter_context(tc.tile_pool(name="ob", bufs=2))
        ppC = _es.enter_context(tc.tile_pool(name="psC", bufs=5, space="PSUM"))
        ppT = _es.enter_context(tc.tile_pool(name="psT", bufs=1, space="PSUM"))
        ppA = _es.enter_context(tc.tile_pool(name="psA", bufs=1, space="PSUM"))
        ppL = _es.enter_context(tc.tile_pool(name="psL", bufs=1, space="PSUM"))

        # ---- constants resident in SBUF
        wcat_sb = pc.tile([P, TT, QKV], f32r, tag="wcat")
        for kt in range(TT):
            nc.sync.dma_start(wcat_sb[:, kt, :], wcat_r[:, kt, :])
        wp_sb = pc.tile([P, 4, D], bf16, tag="wp")
        for ct in range(4):
            nc.sync.dma_start(wp_sb[:, ct, :], wp_r[:, ct, :])
        cos_sb = pc.tile([P, TT, HD // 2], f32, tag="cos")
        nc.sync.dma_start(cos_sb[:], cos_r[:])
        sin_sb = pc.tile([P, TT, HD // 2], f32, tag="sin")
        nc.sync.dma_start(sin_sb[:], sin_r[:])
        mask_sb = pc.tile([P, 4, 512], bf16, tag="mask")
        nc.sync.dma_start(mask_sb[:], maskd[:])
        gain_sb = pc.tile([P, 6], f32, tag="gain")
        nc.sync.dma_start(gain_sb[:], gaind[:])
        idf = pc.tile([P, P], f32, tag="idf")
        make_identity(nc, idf[:])
        idb = pc.tile([P, P], bf16, tag="idb")
        nc.vector.tensor_copy(idb[:], idf[:])
        ones_sb = pc.tile([P, 1], bf16, tag="ones")
        nc.vector.memset(ones_sb[:], 1.0)

        for _rep in range(max(1, loop_n)):
            qT = pb.tile([P, 4, S], bf16, tag="qT")       # [hd, h, tok]
            kT = pb.tile([P, S], bf16, tag="kT")          # [hd, tok]
            vN = pb.tile([P, TT, HD], bf16, tag="vN")     # [tok, tt, e]
            rps = {}

            # ---------------- QKV projection + RMS + RoPE ----------------
            def emit_tp(t):
                # transpose q0..q3,k of tile t into [hd, tok] and evac
                rp = rps.pop(t)
                tpb = ppT.tile([P, 640], bf16, tag="tpb", name="tpb")
                for s in range(5):
                    nc.tensor.transpose(tpb[:, s * P:(s + 1) * P],
                                        rp[:, s, :], idb[:])
                nc.vector.tensor_copy(
                    qT[:, :, t * P:(t + 1) * P],
                    tpb[:, :4 * P].rearrange("p (h x) -> p h x", h=4))
                nc.vector.tensor_copy(kT[:, t * P:(t + 1) * P],
                                      tpb[:, 4 * P:5 * P])

            xts = {}

            def fetch(t):
                if t < TT and t not in xts:
                    xtl = px.tile([P, TT, P], f32r, tag="xt")
                    nc.sync.dma_start(xtl[:], xT_r[:, :, t * P:(t + 1) * P])
                    xts[t] = xtl

            for tt in range(TT):
                fetch(tt)
                fetch(tt + 1)
                fetch(tt + 2)
                xt = xts.pop(tt)

                Ca = ppC.tile([P, 512], f32, tag="C", name="Cq")
                for kt in range(TT):
                    nc.tensor.matmul(Ca[:], xt[:, kt, :], wcat_sb[:, kt, :512],
                                     start=(kt == 0), stop=(kt == TT - 1))
                Cb = ppC.tile([P, 512], f32, tag="C", name="Ckv")
                for kt in range(TT):
                    nc.tensor.matmul(Cb[:, :256], xt[:, kt, :],
                                     wcat_sb[:, kt, 512:768],
                                     start=(kt == 0), stop=(kt == TT - 1))

                # stage q0..q3,k in SBUF f32; v straight to bf16
                stg = pstg.tile([P, 5, P], f32, tag="stg")
                nc.scalar.copy(stg[:, :4, :].rearrange("p s x -> p (s x)"),
                               Ca[:])
                nc.scalar.copy(stg[:, 4, :], Cb[:, :128])
                nc.scalar.copy(vN[:, tt, :], Cb[:, 128:256])

                # rms-norm factors: rs = exp(-.5*ln(ssq/HD+eps)) * gain
                scr = pscr.tile([P, 5, P], f32, tag="scr")
                nc.vector.tensor_tensor(scr[:], stg[:], stg[:], AL.mult)
                ssq = psml.tile([P, 5], f32, tag="ssq")
                nc.vector.tensor_reduce(ssq[:], scr[:], axis=AX.X, op=AL.add)
                ln5 = psml.tile([P, 5], f32, tag="ln5")
                nc.scalar.activation(ln5[:], ssq[:], AF.Ln,
                                     scale=1.0 / HD, bias=gain_sb[:, 5:6])
                rs5 = psml.tile([P, 5], f32, tag="rs5")
                nc.scalar.activation(rs5[:], ln5[:], AF.Exp, scale=-0.5)
                rsg = psml.tile([P, 5], f32, tag="rsg")
                nc.vector.tensor_tensor(rsg[:], rs5[:], gain_sb[:, :5], AL.mult)

                qn = pqn.tile([P, 5, P], f32, tag="qn")
                nc.vector.tensor_tensor(
                    qn[:], stg[:],
                    rsg[:, :, None].to_broadcast([P, 5, P]), AL.mult)

                # rope: o1 = a*cos + b*sin ; o2 = b*cos - a*sin  (bf16 out)
                a = qn[:, :, :HD // 2]
                b2 = qn[:, :, HD // 2:]
                cb = cos_sb[:, None, tt, :].to_broadcast([P, 5, HD // 2])
                sb_ = sin_sb[:, None, tt, :].to_broadcast([P, 5, HD // 2])
                rp = prp.tile([P, 5, P], bf16, tag="rp")
                rps[tt] = rp
                t1 = ptm.tile([P, 5, HD // 2], f32, tag="t1")
                t2 = ptm.tile([P, 5, HD // 2], f32, tag="t2")
                nc.gpsimd.tensor_tensor(t1[:], a, cb, AL.mult)
                nc.vector.tensor_tensor(t2[:], b2, sb_, AL.mult)
                nc.vector.tensor_tensor(rp[:, :, :HD // 2], t1[:], t2[:], AL.add)
                t3 = ptm.tile([P, 5, HD // 2], f32, tag="t3")
                t4 = ptm.tile([P, 5, HD // 2], f32, tag="t4")
                nc.gpsimd.tensor_tensor(t3[:], b2, cb, AL.mult)
                nc.vector.tensor_tensor(t4[:], a, sb_, AL.mult)
                nc.vector.tensor_tensor(rp[:, :, HD // 2:], t3[:], t4[:],
                                        AL.subtract)

                if tt >= 2:
                    emit_tp(tt - 2)

            # ---------------- attention + interleaved out-proj ------------
            # Flat emission stream: sc matmul+exp per k-tile; consumes
            # (ones-mm + AV) lag 3 tiles behind; group finalization and
            # out-proj ride the same queue so the PE never waits.
            import collections
            pending = collections.deque()

            def fin(h, ya, ls, yt):
                def run():
                    rl = prl.tile([1, 512], f32, tag="rl")
                    nc.vector.reciprocal(rl[:], ls[:])
                    rlb = prl.tile([P, 512], f32, tag="rlb")
                    nc.gpsimd.partition_broadcast(rlb[:], rl[0:1, :])
                    nc.vector.tensor_tensor(yt[:, h, :], ya[:], rlb[:], AL.mult)
                return run

            def oproj(g, yt):
                def run():
                    for tl in range(4):
                        ob = pob.tile([P, D], f32, tag="ob")
                        for oc in range(4):
                            Cp = ppC.tile([P, 512], f32, tag="C", name="Cpr")
                            for ct in range(4):
                                nc.tensor.matmul(
                                    Cp[:],
                                    yt[:, ct, tl * P:(tl + 1) * P],
                                    wp_sb[:, ct, oc * 512:(oc + 1) * 512],
                                    start=(ct == 0), stop=(ct == 3))
                            nc.scalar.copy(ob[:, oc * 512:(oc + 1) * 512],
                                           Cp[:])
                        r0 = g * 512 + tl * P
                        nc.sync.dma_start(outd[r0:r0 + P, :], ob[:])
                return run

            def pump(target):
                while len(pending) > target:
                    pending.popleft()()

            for g in range(GROUPS):
                nj = 4 * (g + 1)
                yt = pyT.tile([P, 4, 512], bf16, tag="yt", name=f"yt{g}")
                for h in range(4):
                    ya = ppA.tile([P, 512], f32, tag="ya", name="ya")
                    ls = ppL.tile([1, 512], f32, tag="ls", name="ls")
                    qs = qT[:, h, g * 512:(g + 1) * 512]
                    for jt in range(nj):
                        sc = ppC.tile([P, 512], f32, tag="C", name="sc")
                        nc.tensor.matmul(sc[:], kT[:, jt * P:(jt + 1) * P], qs,
                                         start=True, stop=True)
                        ep = pep.tile([P, 512], bf16, tag="ep")
                        nc.scalar.activation(ep[:], sc[:], AF.Exp)
                        if jt >= 4 * g:
                            nc.vector.tensor_tensor(
                                ep[:], ep[:], mask_sb[:, jt - 4 * g, :],
                                AL.mult)

                        def consume(ep=ep, jt=jt, ya=ya, ls=ls, nj=nj):
                            nc.tensor.matmul(ls[:], ones_sb[:], ep[:],
                                             start=(jt == 0),
                                             stop=(jt == nj - 1))
                            nc.tensor.matmul(ya[:], vN[:, jt, :], ep[:],
                                             start=(jt == 0),
                                             stop=(jt == nj - 1))
                        pending.append(consume)
                        pump(3)
                    pending.append(fin(h, ya, ls, yt))
                    if g == 0 and h in (0, 1):
                        # late qk transposes, covered by attention work
                        pending.append(lambda t=14 + h: emit_tp(t))
                pending.append(oproj(g, yt))
            pump(0)

    nc.compile()
    return nc


def _get_program(loop_n=0):
    key = loop_n
    if key not in _PROG:
        _PROG[key] = _build_program(loop_n)
    return _PROG[key]


def _host_prep(x, Wq, Wk, Wv, Wp, q_gain):
    """Build the 8 per-core input maps. Core c = (batch c//4, kv head c%4)."""
    import ml_dtypes
    bf16 = ml_dtypes.bfloat16

    inv_freq = 1.0 / (ROPE_BASE ** (np.arange(0, HD, 2, dtype=np.float32) / HD))
    freqs = np.arange(S, dtype=np.float32)[:, None] * inv_freq[None, :]
    cos = np.ascontiguousarray(np.cos(freqs).astype(np.float32))
    sin = np.ascontiguousarray(np.sin(freqs).astype(np.float32))

    # causal 0/1 masks for the 4 diagonal-chunk tile variants (il = 0..3):
    # tile rows k (128), group columns q (512): valid iff q >= il*128 + k
    k = np.arange(P)[:, None, None]
    il = np.arange(4)[None, :, None]
    q = np.arange(512)[None, None, :]
    masks = (q >= il * P + k).astype(bf16)               # [128, 4, 512]

    in_maps = []
    for core in range(N_CORES):
        b, kv = divmod(core, 4)
        h0 = 4 * kv
        xT = np.ascontiguousarray(
            x[b].reshape(S, D).T.astype(np.float32))     # [D, S]
        WqT = Wq[h0 * HD:(h0 + 4) * HD, :].T             # [D, 512]
        WkT = Wk[kv * HD:(kv + 1) * HD, :].T             # [D, 128]
        WvT = Wv[kv * HD:(kv + 1) * HD, :].T             # [D, 128]
        wcat = np.ascontiguousarray(
            np.concatenate([WqT, WkT, WvT], axis=1), dtype=np.float32)
        wpT = np.ascontiguousarray(
            Wp[:, h0 * HD:(h0 + 4) * HD].T.astype(bf16))  # [512, D]
        gain = np.tile(np.array(
            [[q_gain[h0] * SCALE, q_gain[h0 + 1] * SCALE,
              q_gain[h0 + 2] * SCALE, q_gain[h0 + 3] * SCALE,
              1.0, EPS]], dtype=np.float32), (P, 1))
        in_maps.append({
            "xT": xT,
            "wcat": wcat,
            "wpd": wpT,
            "cosd": cos,
            "sind": sin,
            "maskd": np.ascontiguousarray(masks),
            "gaind": np.ascontiguousarray(gain),
        })
    return in_maps


def kernel(x, Wq, Wk, Wv, Wp, q_gain):
    from concourse.bass_utils import run_bass_kernel_spmd

    nc = _get_program()
    in_maps = _host_prep(x, Wq, Wk, Wv, Wp, q_gain)
    try:
        res = run_bass_kernel_spmd(nc, in_maps, core_ids=list(range(N_CORES)))
    except Exception:
        # one retry: a previous crashed run can leave the exec unit wedged
        res = run_bass_kernel_spmd(nc, in_maps, core_ids=list(range(N_CORES)))
    out = np.zeros((B, S, D), dtype=np.float32)
    for core in range(N_CORES):
        out[core // 4] += res.results[core]["out"]
    return out
